# revision 3
# baseline (speedup 1.0000x reference)
"""Double-centering kernel for Trainium2 (Bass/Tile), 8-core data parallel.

Computes T = -0.5 * (D - row_mean - col_mean + glob_mean) for
D: [256, 512, 512] f32, sharding the batch dim across 8 NeuronCores
(32 matrices per core, no cross-core communication).

Per-core layout: each [512, 512] matrix is viewed as a [128, 2048] SBUF
tile (partition p holds rows 4p..4p+3), so every DMA is one fully
contiguous 1 MiB transfer.

Per-matrix dataflow (engine balance is the point — DMA is the roofline):
  GPSIMD: S2 = c01+c23, S = S2a+S2b          (partial col sums)
  PE:     C0 = ones[128,128]^T @ S -> PSUM   (col sums bcast to all parts)
  ACT:    Csc = C0/1024 (accum_out gsum)     (= 0.5*col_mean; gsum = g/1024)
  DVE:    v_c = -0.5*D_c (ts 2x mode, accum_out a_c = -0.5*rowsum_c) in place
          g1 = gsum/512; rowterm = -a/512 - g1
          out_c = (v_c + rowterm_c) + Csc    (scalar_tensor_tensor, in place)
  DMA:    1 MiB load (sync HWDGE), 1 MiB store (gpsimd SWDGE)
"""

from contextlib import ExitStack

import numpy as np

import concourse.bacc as bacc
import concourse.tile as tile
from concourse import mybir
from concourse.bass_utils import run_bass_kernel_spmd

N_CORES = 8
B = 256
N = 512
B_LOC = B // N_CORES  # 32 matrices per core
P = 128
CHUNKS = N // P  # 4
FREE = CHUNKS * N  # 2048 elems per partition per matrix

_COMPILED = None
LAST_RESULTS = None  # BassKernelResults of the most recent run (for test harness)


def _build():
    nc = bacc.Bacc("TRN2", target_bir_lowering=False, debug=False)
    d_in = nc.dram_tensor("d_in", [B_LOC, P, FREE], mybir.dt.float32,
                          kind="ExternalInput")
    t_out = nc.dram_tensor("t_out", [B_LOC, P, FREE], mybir.dt.float32,
                           kind="ExternalOutput")
    f32 = mybir.dt.float32

    with tile.TileContext(nc) as tc, ExitStack() as ctx:
        singles = ctx.enter_context(tc.tile_pool(name="singles", bufs=1))
        in_pool = ctx.enter_context(tc.tile_pool(name="in", bufs=8))
        s2_pool = ctx.enter_context(tc.tile_pool(name="s2", bufs=2))
        s_pool = ctx.enter_context(tc.tile_pool(name="s", bufs=3))
        csc_pool = ctx.enter_context(tc.tile_pool(name="csc", bufs=3))
        small = ctx.enter_context(tc.tile_pool(name="small", bufs=4))
        psum = ctx.enter_context(tc.tile_pool(name="psum", bufs=4, space="PSUM"))

        ones_kk = singles.tile([P, P], f32)
        nc.vector.memset(ones_kk[:], 1.0)

        for b in range(B_LOC):
            in_t = in_pool.tile([P, FREE], f32)
            nc.sync.dma_start(out=in_t[:], in_=d_in[b])

            # Partial column sums: S = sum of the 4 row-chunks (GPSIMD).
            s2 = s2_pool.tile([P, 2 * N], f32)
            nc.gpsimd.tensor_add(out=s2[:], in0=in_t[:, :2 * N],
                                 in1=in_t[:, 2 * N:])
            s = s_pool.tile([P, N], f32)
            nc.gpsimd.tensor_add(out=s[:], in0=s2[:, :N], in1=s2[:, N:])

            # Column sums broadcast to all 128 partitions via all-ones matmul.
            c0 = psum.tile([P, N], f32)
            nc.tensor.matmul(out=c0[:], lhsT=ones_kk[:], rhs=s[:],
                             start=True, stop=True)

            # Csc = 0.5*col_mean (SBUF); gsum = g/1024 per partition (ACT).
            csc = csc_pool.tile([P, N], f32)
            gsum = small.tile([P, 1], f32)
            nc.scalar.activation(out=csc[:], in_=c0[:],
                                 func=mybir.ActivationFunctionType.Copy,
                                 bias=0.0, scale=1.0 / 1024.0,
                                 accum_out=gsum[:])

            # v_c = -0.5 * D_c in place; a_c = sum_j v_c = -0.5 * rowsum_c.
            a = small.tile([P, CHUNKS], f32)
            for c in range(CHUNKS):
                sl = slice(c * N, (c + 1) * N)
                nc.vector.tensor_scalar(out=in_t[:, sl], in0=in_t[:, sl],
                                        scalar1=-0.5, scalar2=None,
                                        op0=mybir.AluOpType.mult,
                                        op1=mybir.AluOpType.add,
                                        accum_out=a[:, c:c + 1])

            # rowterm = 0.5*row_mean - 0.5*glob_mean = -a/512 - gsum/512.
            g1 = small.tile([P, 1], f32)
            nc.vector.tensor_scalar_mul(out=g1[:], in0=gsum[:],
                                        scalar1=1.0 / 512.0)
            rowterm = small.tile([P, CHUNKS], f32)
            nc.vector.tensor_scalar(out=rowterm[:], in0=a[:],
                                    scalar1=-1.0 / 512.0, scalar2=g1[:],
                                    op0=mybir.AluOpType.mult,
                                    op1=mybir.AluOpType.subtract)

            # out_c = (v_c + rowterm_c) + Csc, fused and in place.
            for c in range(CHUNKS):
                sl = slice(c * N, (c + 1) * N)
                nc.vector.scalar_tensor_tensor(out=in_t[:, sl],
                                               in0=in_t[:, sl],
                                               scalar=rowterm[:, c:c + 1],
                                               in1=csc[:],
                                               op0=mybir.AluOpType.add,
                                               op1=mybir.AluOpType.add)

            nc.gpsimd.dma_start(out=t_out[b], in_=in_t[:])

    nc.compile()
    return nc


def _get_nc():
    global _COMPILED
    if _COMPILED is None:
        _COMPILED = _build()
    return _COMPILED


def kernel(D: np.ndarray) -> np.ndarray:
    global LAST_RESULTS
    D = np.ascontiguousarray(np.asarray(D), dtype=np.float32)
    assert D.shape == (B, N, N), D.shape
    shards = D.reshape(N_CORES, B_LOC, P, FREE)
    nc = _get_nc()
    in_maps = [{"d_in": shards[i]} for i in range(N_CORES)]
    res = run_bass_kernel_spmd(nc, in_maps, core_ids=list(range(N_CORES)))
    LAST_RESULTS = res
    out = np.stack([res.results[i]["t_out"] for i in range(N_CORES)])
    return out.reshape(B, N, N).astype(np.float32, copy=False)


# revision 4
# speedup vs baseline: 1.0779x; 1.0779x over previous
"""Double-centering kernel for Trainium2 (Bass/Tile), 8-core data parallel.

Computes T = -0.5 * (D - row_mean - col_mean + glob_mean) for
D: [256, 512, 512] f32, sharding the batch dim across 8 NeuronCores
(32 matrices per core, no cross-core communication).

Per-core layout: each [512, 512] matrix is viewed as a [128, 2048] SBUF
tile (partition p holds rows 4p..4p+3), so every DMA is one fully
contiguous 1 MiB transfer.

Per-matrix dataflow (engine balance is the point — DMA is the roofline):
  GPSIMD: S2 = c01+c23, S = S2a+S2b          (partial col sums)
  PE:     C0 = ones[128,128]^T @ S -> PSUM   (col sums bcast to all parts)
  ACT:    Csc = C0/1024 (accum_out gsum)     (= 0.5*col_mean; gsum = g/1024)
  DVE:    v_c = -0.5*D_c (ts 2x mode, accum_out a_c = -0.5*rowsum_c) in place
          g1 = gsum/512; rowterm = -a/512 - g1
          out_c = (v_c + rowterm_c) + Csc    (scalar_tensor_tensor, in place)
  DMA:    1 MiB load (sync HWDGE), 1 MiB store (gpsimd SWDGE)
"""

from contextlib import ExitStack

import numpy as np

import concourse.bacc as bacc
import concourse.tile as tile
from concourse import mybir
from concourse.bass_utils import run_bass_kernel_spmd

N_CORES = 8
B = 256
N = 512
B_LOC = B // N_CORES  # 32 matrices per core
P = 128
CHUNKS = N // P  # 4
FREE = CHUNKS * N  # 2048 elems per partition per matrix

_COMPILED = None
LAST_RESULTS = None  # BassKernelResults of the most recent run (for test harness)


def _build():
    nc = bacc.Bacc("TRN2", target_bir_lowering=False, debug=False)
    d_in = nc.dram_tensor("d_in", [B_LOC, P, FREE], mybir.dt.float32,
                          kind="ExternalInput")
    t_out = nc.dram_tensor("t_out", [B_LOC, P, FREE], mybir.dt.float32,
                           kind="ExternalOutput")
    f32 = mybir.dt.float32

    with tile.TileContext(nc) as tc, ExitStack() as ctx:
        singles = ctx.enter_context(tc.tile_pool(name="singles", bufs=1))
        in_pool = ctx.enter_context(tc.tile_pool(name="in", bufs=8))
        s2_pool = ctx.enter_context(tc.tile_pool(name="s2", bufs=2))
        s_pool = ctx.enter_context(tc.tile_pool(name="s", bufs=3))
        csc_pool = ctx.enter_context(tc.tile_pool(name="csc", bufs=3))
        small = ctx.enter_context(tc.tile_pool(name="small", bufs=4))
        psum = ctx.enter_context(tc.tile_pool(name="psum", bufs=4, space="PSUM"))

        ones_kk = singles.tile([P, P], f32)
        nc.vector.memset(ones_kk[:], 1.0)

        for b in range(B_LOC):
            in_t = in_pool.tile([P, FREE], f32)
            nc.sync.dma_start(out=in_t[:], in_=d_in[b])

            # Partial column sums: S = sum of the 4 row-chunks (GPSIMD).
            s2 = s2_pool.tile([P, 2 * N], f32)
            nc.gpsimd.tensor_add(out=s2[:], in0=in_t[:, :2 * N],
                                 in1=in_t[:, 2 * N:])
            s = s_pool.tile([P, N], f32)
            nc.gpsimd.tensor_add(out=s[:], in0=s2[:, :N], in1=s2[:, N:])

            # Column sums broadcast to all 128 partitions via all-ones matmul.
            c0 = psum.tile([P, N], f32)
            nc.tensor.matmul(out=c0[:], lhsT=ones_kk[:], rhs=s[:],
                             start=True, stop=True)

            # v_c = -0.5 * D_c in place (ACT); a_c = sum_j v_c = -0.5*rowsum_c.
            a = small.tile([P, CHUNKS], f32)
            for c in range(CHUNKS):
                sl = slice(c * N, (c + 1) * N)
                nc.scalar.activation(out=in_t[:, sl], in_=in_t[:, sl],
                                     func=mybir.ActivationFunctionType.Copy,
                                     bias=0.0, scale=-0.5,
                                     accum_out=a[:, c:c + 1])

            # Csc = 0.5*col_mean (SBUF); gsum = g/1024 per partition (ACT).
            csc = csc_pool.tile([P, N], f32)
            gsum = small.tile([P, 1], f32)
            nc.scalar.activation(out=csc[:], in_=c0[:],
                                 func=mybir.ActivationFunctionType.Copy,
                                 bias=0.0, scale=1.0 / 1024.0,
                                 accum_out=gsum[:])

            # rowterm = 0.5*row_mean - 0.5*glob_mean = -a/512 - gsum/512.
            g1 = small.tile([P, 1], f32)
            nc.vector.tensor_scalar_mul(out=g1[:], in0=gsum[:],
                                        scalar1=1.0 / 512.0)
            rowterm = small.tile([P, CHUNKS], f32)
            nc.vector.tensor_scalar(out=rowterm[:], in0=a[:],
                                    scalar1=-1.0 / 512.0, scalar2=g1[:],
                                    op0=mybir.AluOpType.mult,
                                    op1=mybir.AluOpType.subtract)

            # out_c = (v_c + rowterm_c) + Csc, fused and in place.
            for c in range(CHUNKS):
                sl = slice(c * N, (c + 1) * N)
                nc.vector.scalar_tensor_tensor(out=in_t[:, sl],
                                               in0=in_t[:, sl],
                                               scalar=rowterm[:, c:c + 1],
                                               in1=csc[:],
                                               op0=mybir.AluOpType.add,
                                               op1=mybir.AluOpType.add)

            nc.gpsimd.dma_start(out=t_out[b], in_=in_t[:])

    nc.compile()
    return nc


def _get_nc():
    global _COMPILED
    if _COMPILED is None:
        _COMPILED = _build()
    return _COMPILED


def kernel(D: np.ndarray) -> np.ndarray:
    global LAST_RESULTS
    D = np.ascontiguousarray(np.asarray(D), dtype=np.float32)
    assert D.shape == (B, N, N), D.shape
    shards = D.reshape(N_CORES, B_LOC, P, FREE)
    nc = _get_nc()
    in_maps = [{"d_in": shards[i]} for i in range(N_CORES)]
    res = run_bass_kernel_spmd(nc, in_maps, core_ids=list(range(N_CORES)))
    LAST_RESULTS = res
    out = np.stack([res.results[i]["t_out"] for i in range(N_CORES)])
    return out.reshape(B, N, N).astype(np.float32, copy=False)
